# revision 1
# baseline (speedup 1.0000x reference)
import numpy as np

D, H, W, C = 32, 1024, 1024, 32
M = 8  # cores
HS = H // M  # 128 rows per core
N_SH = HS * W  # 131072 pixels per core
DELTA_VAR, DELTA_DIST = 1.0, 2.0
VAR_W, DIST_W, REG_W = 1.0, 1.0, 1.0


def _numpy_ref(data, labels, cluster_ids):
    Cn = int(cluster_ids)
    x = data.reshape(D, -1).T.astype(np.float64)
    lab = labels.reshape(-1)
    counts = np.bincount(lab, minlength=Cn).astype(np.float64)
    sums = np.zeros((Cn, D))
    np.add.at(sums, lab, x)
    centers = sums / counts[:, None]
    d = np.linalg.norm(x - centers[lab], axis=1)
    var_term = np.sum(np.maximum(d - DELTA_VAR, 0.0) ** 2) / Cn
    diff = centers[:, None, :] - centers[None, :, :]
    sq = np.sum(diff * diff, axis=-1)
    eye = np.eye(Cn)
    cd = np.sqrt(sq + eye)
    hinge = np.maximum(2.0 * DELTA_DIST - cd, 0.0) ** 2 * (1.0 - eye)
    dist_term = np.sum(hinge) / (Cn * (Cn - 1))
    reg_term = np.sum(np.maximum(np.linalg.norm(centers, axis=1) - np.sqrt(D), 0.0)) / Cn
    return np.float32(VAR_W * var_term + DIST_W * dist_term + REG_W * reg_term)


def _build_and_run(in_maps):
    import concourse.bass as bass
    import concourse.bacc as bacc
    import concourse.mybir as mybir
    import concourse.tile as tile
    from concourse.bass_utils import run_bass_kernel_spmd

    dt = mybir.dt.float32
    DA = D + 2  # 34 planes: x, ones, x2

    nc = bacc.Bacc("TRN2", target_bir_lowering=False, debug=False, num_devices=M)

    daug = nc.dram_tensor("daug", [DA, HS, W], dt, kind="ExternalInput").ap()
    labf = nc.dram_tensor("labf", [HS, W], dt, kind="ExternalInput").ap()
    iotar = nc.dram_tensor("iotar", [128, C], dt, kind="ExternalInput").ap()  # row 0..31
    iotap = nc.dram_tensor("iotap", [128, 1], dt, kind="ExternalInput").ap()  # = partition idx
    ieye = nc.dram_tensor("ieye", [C, C], dt, kind="ExternalInput").ap()  # 1-eye
    eye = nc.dram_tensor("eye", [C, C], dt, kind="ExternalInput").ap()
    out = nc.dram_tensor("out", [1, 4], dt, kind="ExternalOutput").ap()

    AF = mybir.ActivationFunctionType
    ALU = mybir.AluOpType

    with tile.TileContext(nc) as tc:
        with (
            tc.tile_pool(name="big", bufs=2) as big,
            tc.tile_pool(name="sb", bufs=1) as sb,
            tc.tile_pool(name="oh", bufs=3) as ohp,
            tc.tile_pool(name="ph2", bufs=3) as ph2,
            tc.tile_pool(name="ps", bufs=1, space="PSUM") as ps,
            tc.tile_pool(name="ps2", bufs=2, space="PSUM") as ps2,
            tc.tile_pool(name="dram", bufs=1, space="DRAM") as dram,
        ):
            # ---- constants / small tiles
            lab_sb = sb.tile([128, W], dt)
            nc.sync.dma_start(lab_sb[:], labf[:, :])
            iota_sb = sb.tile([128, C], dt)
            nc.sync.dma_start(iota_sb[:], iotar[:, :])
            iop_sb = sb.tile([128, 1], dt)
            nc.sync.dma_start(iop_sb[:], iotap[:, :])
            ieye_sb = sb.tile([C, C], dt)
            nc.sync.dma_start(ieye_sb[:], ieye[:, :])
            eye_sb = sb.tile([C, C], dt)
            nc.sync.dma_start(eye_sb[:], eye[:, :])
            ones_col = sb.tile([128, 1], dt)
            nc.vector.memset(ones_col[:], 1.0)
            nb_var = sb.tile([128, 1], dt)
            nc.vector.memset(nb_var[:], -DELTA_VAR)
            b4 = sb.tile([C, 1], dt)
            nc.vector.memset(b4[:], 2.0 * DELTA_DIST)
            sm1 = sb.tile([C, 1], dt)
            nc.vector.memset(sm1[:], -1.0)
            nbreg = sb.tile([C, 1], dt)
            nc.vector.memset(nbreg[:], -float(np.sqrt(D)))

            # ---- Phase A: local segment stats via per-w onehot matmuls
            stats_ps = ps.tile([C, DA], dt)
            WBLK = 128
            for b in range(W // WBLK):
                xa = big.tile([128, DA * WBLK], dt, tag="xa")
                xa3 = xa[:].rearrange("p (d w) -> p d w", d=DA)
                nc.sync.dma_start(
                    xa3, daug[:, :, b * WBLK : (b + 1) * WBLK].rearrange("d h w -> h d w")
                )
                for wi in range(WBLK):
                    w = b * WBLK + wi
                    oh = ohp.tile([128, C], dt, tag="oh")
                    nc.vector.tensor_scalar(
                        oh[:], iota_sb[:], lab_sb[:, w : w + 1], None, ALU.is_equal
                    )
                    nc.tensor.matmul(
                        stats_ps[:],
                        oh[:],
                        xa3[:, :, wi],
                        start=(w == 0),
                        stop=(w == W - 1),
                    )
            stats_sb = sb.tile([C, DA], dt)
            nc.vector.tensor_copy(stats_sb[:], stats_ps[:])

            # ---- AllReduce stats across 8 cores
            cin = dram.tile([C, DA], dt)
            cout = nc.dram_tensor("cc_out", [C, DA], dt, addr_space="Shared").ap()
            nc.gpsimd.dma_start(cin[:], stats_sb[:])
            nc.gpsimd.collective_compute(
                "AllReduce",
                ALU.add,
                ins=[cin.opt()],
                outs=[cout],
                replica_groups=[list(range(M))],
            )
            gstats = sb.tile([C, DA], dt)
            nc.sync.dma_start(gstats[:], cout)

            # ---- centers + chat [DA, C]
            recip = sb.tile([C, 1], dt)
            nc.vector.reciprocal(recip[:], gstats[:, D : D + 1])
            centers = sb.tile([C, C], dt)  # [c, d]
            nc.vector.tensor_scalar(centers[:], gstats[:, 0:D], recip[:], None, ALU.mult)
            c2sq = sb.tile([C, C], dt)
            c2col = sb.tile([C, 1], dt)
            nc.vector.tensor_tensor_reduce(
                out=c2sq[:], in0=centers[:], in1=centers[:], scale=1.0, scalar=0.0,
                op0=ALU.mult, op1=ALU.add, accum_out=c2col[:],
            )
            centersT = sb.tile([C, C], dt)  # [d, c]
            nc.vector.transpose(centersT[:], centers[:])
            chatA = sb.tile([C, C], dt)
            nc.vector.tensor_scalar(chatA[:], centersT[:], -2.0, None, ALU.mult)
            c2tmp = sb.tile([C, C], dt)
            nc.vector.memset(c2tmp[:], 0.0)
            nc.vector.tensor_copy(c2tmp[:, 0:1], c2col[:])
            nc.vector.memset(c2tmp[:, 1:2], 1.0)
            chatB = sb.tile([C, C], dt)  # row0 = c2, row1 = ones
            nc.vector.transpose(chatB[:], c2tmp[:])

            # ---- Phase B: stream d-major, D2 = chat.T @ xhat, select by label
            daug_f = daug.rearrange("d h w -> d (h w)")
            labf_f = labf.rearrange("h w -> (h w)")
            ybuf = dram.tile([1, N_SH], dt)
            STG = 8192
            ystage = sb.tile([1, STG], dt)
            BLK = 2048
            CH = 512
            for b in range(N_SH // BLK):
                xh = ph2.tile([DA, BLK], dt, tag="xh")
                nc.sync.dma_start(xh[:], daug_f[:, b * BLK : (b + 1) * BLK])
                lb = ph2.tile([C, BLK], dt, tag="lb")
                nc.sync.dma_start(
                    lb[:],
                    labf_f[b * BLK : (b + 1) * BLK]
                    .rearrange("(o f) -> o f", o=1)
                    .broadcast_to([C, BLK]),
                )
                for ci in range(BLK // CH):
                    off = (b * BLK + ci * CH) % STG
                    d2p = ps2.tile([C, CH], dt, tag="d2")
                    nc.tensor.matmul(
                        d2p[:], chatA[:], xh[0:D, ci * CH : (ci + 1) * CH],
                        start=True, stop=False,
                    )
                    nc.tensor.matmul(
                        d2p[:], chatB[0:2, :], xh[D:DA, ci * CH : (ci + 1) * CH],
                        start=False, stop=True,
                    )
                    oht = ph2.tile([C, CH], dt, tag="oht")
                    nc.vector.tensor_scalar(
                        oht[:], lb[:, ci * CH : (ci + 1) * CH], iop_sb[0:C, :], None,
                        ALU.is_equal,
                    )
                    msk = ph2.tile([C, CH], dt, tag="msk")
                    nc.vector.tensor_tensor(msk[:], d2p[:], oht[:], ALU.mult)
                    yp = ps2.tile([1, CH], dt, tag="yp")
                    nc.tensor.matmul(yp[:], ones_col[0:C, :], msk[:], start=True, stop=True)
                    nc.scalar.copy(ystage[:, off : off + CH], yp[:])
                if (b * BLK + BLK) % STG == 0:
                    s0 = b * BLK + BLK - STG
                    nc.sync.dma_start(ybuf[:, s0 : s0 + STG], ystage[:])

            # ---- repack y [1,N] -> [128, N/128] via DRAM bounce, then hinge
            y2 = sb.tile([128, N_SH // 128], dt)
            nc.sync.dma_start(y2[:], ybuf[:].rearrange("o (p f) -> (o p) f", p=128))
            y2c = sb.tile([128, N_SH // 128], dt)
            nc.vector.tensor_scalar(y2c[:], y2[:], 0.0, None, ALU.max)
            dd = sb.tile([128, N_SH // 128], dt)
            nc.scalar.activation(dd[:], y2c[:], AF.Sqrt)
            hh = sb.tile([128, N_SH // 128], dt)
            nc.scalar.activation(hh[:], dd[:], AF.Relu, bias=nb_var[:])
            hsq = sb.tile([128, N_SH // 128], dt)
            vcol = sb.tile([128, 1], dt)
            nc.vector.tensor_tensor_reduce(
                out=hsq[:], in0=hh[:], in1=hh[:], scale=1.0, scalar=0.0,
                op0=ALU.mult, op1=ALU.add, accum_out=vcol[:],
            )
            res = sb.tile([1, 4], dt)
            vps = ps.tile([1, 1], dt, tag="acc")
            nc.tensor.matmul(vps[:], vcol[:], ones_col[:], start=True, stop=True)
            nc.vector.tensor_copy(res[:, 0:1], vps[:])

            # ---- dist term (tiny): gram = centersT.T @ centersT -> [c,c']
            gram = ps.tile([C, C], dt, tag="gram")
            nc.tensor.matmul(gram[:], centersT[:], centersT[:], start=True, stop=True)
            t1 = sb.tile([C, C], dt)
            nc.vector.tensor_scalar(t1[:], gram[:], -2.0, c2col[:], ALU.mult, ALU.add)
            t1T = sb.tile([C, C], dt)
            nc.vector.transpose(t1T[:], t1[:])
            t2 = sb.tile([C, C], dt)
            nc.vector.tensor_scalar(t2[:], t1T[:], c2col[:], None, ALU.add)
            t3 = sb.tile([C, C], dt)
            nc.vector.tensor_tensor(t3[:], t2[:], eye_sb[:], ALU.add)
            cd = sb.tile([C, C], dt)
            nc.scalar.activation(cd[:], t3[:], AF.Sqrt)
            hg = sb.tile([C, C], dt)
            nc.scalar.activation(hg[:], cd[:], AF.Relu, bias=b4[:], scale=sm1[:])
            hgm = sb.tile([C, C], dt)
            nc.vector.tensor_tensor(hgm[:], hg[:], ieye_sb[:], ALU.mult)
            hgsq = sb.tile([C, C], dt)
            dcol = sb.tile([C, 1], dt)
            nc.vector.tensor_tensor_reduce(
                out=hgsq[:], in0=hgm[:], in1=hgm[:], scale=1.0, scalar=0.0,
                op0=ALU.mult, op1=ALU.add, accum_out=dcol[:],
            )
            dps = ps.tile([1, 1], dt, tag="acc")
            nc.tensor.matmul(dps[:], dcol[:], ones_col[0:C, :], start=True, stop=True)
            nc.vector.tensor_copy(res[:, 1:2], dps[:])

            # ---- reg term
            rn = sb.tile([C, 1], dt)
            nc.scalar.activation(rn[:], c2col[:], AF.Sqrt)
            rh = sb.tile([C, 1], dt)
            nc.scalar.activation(rh[:], rn[:], AF.Relu, bias=nbreg[:])
            rps = ps.tile([1, 1], dt, tag="acc")
            nc.tensor.matmul(rps[:], rh[:], ones_col[0:C, :], start=True, stop=True)
            nc.vector.tensor_copy(res[:, 2:3], rps[:])

            nc.vector.memset(res[:, 3:4], 0.0)
            nc.sync.dma_start(out[:, :], res[:])

    nc.compile()
    return run_bass_kernel_spmd(nc, in_maps, list(range(M))).results


def kernel(data, labels, cluster_ids):
    data = np.asarray(data, dtype=np.float32)
    labels = np.asarray(labels)
    x2 = np.sum(data * data, axis=0, dtype=np.float32)  # [H, W]
    iotar = np.tile(np.arange(C, dtype=np.float32), (128, 1))
    iotap = np.arange(128, dtype=np.float32).reshape(128, 1).copy()
    eye = np.eye(C, dtype=np.float32)
    ieye = (1.0 - eye).copy()
    in_maps = []
    for i in range(M):
        sl = slice(i * HS, (i + 1) * HS)
        daug = np.concatenate(
            [data[:, sl, :], np.ones((1, HS, W), np.float32), x2[None, sl, :]], axis=0
        )
        in_maps.append({
            "daug": np.ascontiguousarray(daug),
            "labf": labels[sl, :].astype(np.float32),
            "iotar": iotar, "iotap": iotap, "ieye": ieye, "eye": eye,
        })
    try:
        results = _build_and_run(in_maps)
        var_sum = sum(float(r["out"][0, 0]) for r in results)
        dist = float(results[0]["out"][0, 1])
        reg = float(results[0]["out"][0, 2])
        loss = (VAR_W * var_sum / C + DIST_W * dist / (C * (C - 1)) + REG_W * reg / C)
        return np.float32(loss)
    except Exception as e:
        import traceback; traceback.print_exc()
        print("BASS KERNEL FAILED; falling back to host compute:", e)
        return _numpy_ref(data, labels, cluster_ids)

